# revision 35
# baseline (speedup 1.0000x reference)
"""Single-head causal self-attention on 8 Trainium2 NeuronCores.

Problem: x[8, 4096, 1024], Wq/Wk/Wv[1024, 128] ->
  out[b] = softmax(causal((x[b] @ Wq) @ (x[b] @ Wk)^T / sqrt(128))) @ (x[b] @ Wv)

Sharding: data-parallel over batch -- each of the 8 cores handles one batch
element. Inputs are fed per-core as xT = x[b].T (layout prep on host) so the
contraction dim C lands on SBUF partitions.

Per-core kernel (T=4096, C=1024, HS=128), all matmuls bf16 (1 col/cycle).

Structure: software-pipelined weave. The attention k-loop of q-group g is
paced by the ACT engine (exp ~1.3us per 1024-col tile) which leaves the PE
~40% idle; the QKV projection matmuls of t-group g+1 are emitted BETWEEN
k-iterations of group g to fill that idle time. Projections use two
ping-pong PSUM banks (sequential q/k/v passes + v-transposes), attention
uses 2x[P,1024] score banks + one [P,1024] output accumulator; 8 banks
total.

Per k-iteration: scores s^T[kv,q] via 2 matmuls (exact causal start), one
1024-wide exp on ACT -> bf16 pt, diagonal 128-block masked with an
upper-tri mask (DVE), denominator accumulated in two alternating bf16
chains (DVE), PV accumulated into PSUM with a 1-iteration lookahead
(scores(k) before PV(k-1)). Per-group epilogue: ones-matmul partition
reduce of both chains, fast reciprocal, DRAM-broadcast of the per-column
reciprocal; the normalize+store tail is deferred into the next group.
"""

import numpy as np
import ml_dtypes

import concourse.bass as bass
import concourse.tile as tile
from concourse import bacc, mybir
from concourse.bass_utils import run_bass_kernel_spmd

B, T, C, HS = 8, 4096, 1024, 128
P = 128
NCORES = 8
CCH = C // P            # 8 c-chunks
NT = T // P             # 32 t/kv blocks of 128
TG = T // 1024          # 4 t-groups of 1024
QG = T // 1024          # 4 q-groups of 1024
SCALE = float(HS) ** -0.5

f32 = mybir.dt.float32
f32r = mybir.dt.float32r
bf16 = mybir.dt.bfloat16
EXP = mybir.ActivationFunctionType.Exp

_NC = None


def build_program():
    nc = bacc.Bacc()
    xT = nc.declare_dram_parameter("xT", [C, T], bf16, isOutput=False)
    Wq = nc.declare_dram_parameter("Wq", [C, HS], bf16, isOutput=False)
    Wk = nc.declare_dram_parameter("Wk", [C, HS], bf16, isOutput=False)
    Wv = nc.declare_dram_parameter("Wv", [C, HS], bf16, isOutput=False)
    # host-provided constants: [ones(2) | identity(128) | trimask(128)]
    aux = nc.declare_dram_parameter("aux", [P, 258], f32, isOutput=False)
    outT = nc.declare_dram_parameter("outT", [HS, T], f32, isOutput=True)
    rscratch = nc.dram_tensor("rscratch", [QG, 1024], f32)

    xT_r = xT[:].rearrange("(j p) t -> p j t", p=P)
    w_views = [w[:].rearrange("(j p) d -> p j d", p=P) for w in (Wq, Wk, Wv)]

    with tile.TileContext(nc) as tc:
        with (
            tc.tile_pool(name="consts", bufs=1) as consts,
            tc.tile_pool(name="big", bufs=1) as big,
            tc.tile_pool(name="xin", bufs=2) as xin,
            tc.tile_pool(name="vtp", bufs=2) as vtp,
            tc.tile_pool(name="ptp", bufs=6) as ptp,
            tc.tile_pool(name="accp", bufs=2) as accp,
            tc.tile_pool(name="ocnp", bufs=2) as ocnp,
            tc.tile_pool(name="recipp", bufs=2) as recipp,
            tc.tile_pool(name="ps_pj", bufs=1, space="PSUM") as ps_pj,
            tc.tile_pool(name="ps_s", bufs=2, space="PSUM") as ps_s,
            tc.tile_pool(name="ps_o", bufs=1, space="PSUM") as ps_o,
        ):
            # Wq rides first on the sync queue (the q-pass only needs Wq and
            # x chunk 0); Wk/Wv follow interleaved with the x stream. The
            # scalar queue starts late (ACT table load), so avoid it here.
            w_sb = [consts.tile([P, CCH, HS], bf16, tag=f"w{i}", name=f"w{i}")
                    for i in range(3)]
            nc.sync.dma_start(out=w_sb[0][:], in_=w_views[0])

            aux_sb = consts.tile([P, 258], f32r)
            nc.gpsimd.dma_start(out=aux_sb[:], in_=aux[:].bitcast(f32r))
            ones = aux_sb[:, 0:2]
            ident = aux_sb[:, 2:130]
            trimask = aux_sb[:, 130:258]

            trimask_b = consts.tile([P, P], bf16)
            nc.vector.tensor_copy(trimask_b[:], trimask)
            ident_b = consts.tile([P, P], bf16)
            nc.vector.tensor_copy(ident_b[:], ident)
            ones_b = consts.tile([P, 2], bf16)
            nc.vector.tensor_copy(ones_b[:], ones)

            qT = big.tile([P, T], bf16, tag="qT")   # [d, t]
            kT = big.tile([P, T], bf16, tag="kT")   # [d, t]
            vS = big.tile([P, NT, HS], bf16, tag="vS")  # [t-in-block, block, d]

            def dma_xin(tg):
                tg0 = 1024 * tg
                xts = [xin.tile([P, 1024], bf16, tag=f"xt{j}", name=f"xt{j}")
                       for j in range(CCH)]
                for j in range(CCH):
                    # t-group 0 gates the cold start: split it across the
                    # sync and (then-idle) gpsimd queues
                    eng = nc.gpsimd if (tg == 0 and j % 2 == 1) else nc.sync
                    eng.dma_start(out=xts[j][:],
                                  in_=xT_r[:, j, tg0:tg0 + 1024])
                    if tg == 0 and j == 0:
                        nc.gpsimd.dma_start(out=w_sb[1][:], in_=w_views[1])
                        nc.sync.dma_start(out=w_sb[2][:], in_=w_views[2])
                return xts

            def proj_items(tg, xts):
                """Generator of closures: projection work for t-group tg,
                emitted piecewise between attention iterations."""
                for h in range(2):
                    t0 = 1024 * tg + 512 * h
                    hs = slice(512 * h, 512 * (h + 1))
                    pA = ps_pj.tile([P, 512], f32, tag="pjA", name="pjA")
                    pB = ps_pj.tile([P, 512], f32, tag="pjB", name="pjB")

                    def mk_mm(ps, i, j):
                        def f():
                            nc.tensor.matmul(
                                ps[:], lhsT=w_sb[i][:, j, :],
                                rhs=xts[j][:, hs],
                                start=(j == 0), stop=(j == CCH - 1))
                        return f
                    for j in range(CCH):
                        yield mk_mm(pA, 0, j)
                    for j in range(CCH):
                        yield mk_mm(pB, 1, j)
                    yield lambda pA=pA, t0=t0: nc.vector.tensor_copy(
                        qT[:, t0:t0 + 512], pA[:])
                    for j in range(CCH):
                        yield mk_mm(pA, 2, j)
                    yield lambda pB=pB, t0=t0: nc.vector.tensor_copy(
                        kT[:, t0:t0 + 512], pB[:])

                    vt = vtp.tile([P, 512], bf16)
                    yield lambda pA=pA, vt=vt: nc.vector.tensor_copy(
                        vt[:], pA[:])

                    def mk_tr(vt, tg, h, m):
                        def f():
                            tp = ps_pj.tile([P, P], bf16, tag="pjB",
                                            name="pjB_tr")
                            nc.tensor.transpose(
                                tp[:], vt[:, 128 * m:128 * (m + 1)], ident_b)
                            nc.vector.tensor_copy(
                                vS[:, 8 * tg + 4 * h + m, :], tp[:])
                        return f
                    for m in range(4):
                        yield mk_tr(vt, tg, h, m)

            def drain(it, n=-1):
                """Emit up to n items from iterator (all if n < 0)."""
                k = 0
                for f in it:
                    f()
                    k += 1
                    if 0 <= n <= k:
                        break
                return k

            # -------- upfront: project t-group 0 --------
            xts0 = dma_xin(0)
            items = proj_items(0, xts0)
            drain(items)

            # -------- woven attention + next projections --------
            pending = None  # deferred normalize+store of the previous group
            for g in range(QG):
                q0 = 1024 * g
                if g + 1 < TG:
                    xts = dma_xin(g + 1)
                    items = proj_items(g + 1, xts)
                    n_items = 60
                else:
                    items = iter(())
                    n_items = 0
                o_ps = ps_o.tile([P, 1024], f32)
                acc_a = accp.tile([P, 1024], bf16, tag="acc_a", name="acc_a")
                acc_b = accp.tile([P, 1024], bf16, tag="acc_b", name="acc_b")
                nkv = 8 * (g + 1)
                per_k = -(-n_items // max(nkv - 1, 1))  # ceil
                prev = None  # (k, vstart, pt_tile)

                def emit_pv(k, vstart, pt):
                    for c in range(2):
                        cq = 512 * c
                        lc = max(0, vstart - cq)
                        if lc >= 512:
                            continue
                        nc.tensor.matmul(
                            o_ps[:, cq + lc:cq + 512],
                            lhsT=vS[:, k, :], rhs=pt[:, cq + lc:cq + 512],
                            start=(k == 0), stop=(k == 8 * g + 4 * c + 3),
                        )

                lastg = {}  # ocu/ocn tiles for the last group's split epilogue

                def emit_half_epi(hh):
                    # denominator+normalize+store for 512 columns of the LAST
                    # group; half 0 runs while k=8g+4..nkv-1 still compute
                    cs = slice(512 * hh, 512 * (hh + 1))
                    drt = ps_s.tile([P, 1024], f32, tag="s", name="drt")
                    nc.tensor.matmul(drt[0:2, 0:512], lhsT=ones_b,
                                     rhs=acc_a[:, cs], start=True, stop=False)
                    nc.tensor.matmul(drt[0:2, 0:512], lhsT=ones_b,
                                     rhs=acc_b[:, cs], start=False, stop=True)
                    rT = recipp.tile([1, 512], f32, tag=f"rT{hh}",
                                     name=f"rT{hh}")
                    nc.vector.reciprocal_approx_fast(rT[:], drt[0:1, 0:512])
                    nc.gpsimd.dma_start(out=rscratch[g:g + 1, cs], in_=rT[:])
                    rs = rscratch[g:g + 1, cs]
                    rs_b = bass.AP(tensor=rs.tensor, offset=rs.offset,
                                   ap=[[0, P], rs.ap[-1]])
                    rB = recipp.tile([P, 512], f32, tag=f"rB{hh}",
                                     name=f"rB{hh}")
                    nc.gpsimd.dma_start(out=rB[:], in_=rs_b)
                    if not lastg:
                        lastg["ocu"] = ocnp.tile([P, 1024], f32, tag="ocu",
                                                 name="ocu")
                        lastg["ocn"] = ocnp.tile([P, 1024], f32, tag="ocn",
                                                 name="ocn")
                    nc.vector.tensor_copy(lastg["ocu"][:, cs], o_ps[:, cs])
                    nc.vector.tensor_mul(lastg["ocn"][:, cs],
                                         lastg["ocu"][:, cs], rB[:])
                    nc.sync.dma_start(
                        out=outT[:, 1024 * g + 512 * hh:
                                 1024 * g + 512 * (hh + 1)],
                        in_=lastg["ocn"][:, cs])

                for k in range(nkv):
                    vstart = max(0, 128 * k - q0)
                    sp = ps_s.tile([P, 1024], f32, tag="s", name="sp")
                    for c in range(2):
                        cq = 512 * c
                        lc = max(0, vstart - cq)
                        if lc >= 512:
                            continue  # chunk fully above diagonal
                        nc.tensor.matmul(
                            sp[:, cq + lc:cq + 512],
                            lhsT=kT[:, 128 * k:128 * (k + 1)],
                            rhs=qT[:, q0 + cq + lc:q0 + cq + 512],
                            start=True, stop=True,
                        )
                    pt = ptp.tile([P, 1024], bf16, tag="pt", name="pt")
                    nc.scalar.activation(
                        pt[:, vstart:1024], sp[:, vstart:1024], EXP,
                        scale=SCALE)
                    if k >= 8 * g:  # diagonal block: mask kv > q
                        nc.vector.tensor_mul(
                            pt[:, vstart:vstart + 128],
                            pt[:, vstart:vstart + 128], trimask_b[:])
                    # denominator partial sums: two alternating bf16 chains
                    acc_t = acc_a if k % 2 == 0 else acc_b
                    if k == 0:
                        nc.vector.tensor_copy(acc_t[:], pt[:])
                    elif k == 1:
                        nc.vector.tensor_copy(
                            acc_t[:, vstart:1024], pt[:, vstart:1024])
                        if g == 0:
                            nc.vector.memset(
                                acc_t[:, 0:128].bitcast(f32), 0.0)
                    else:
                        nc.vector.tensor_add(
                            acc_t[:, vstart:1024], acc_t[:, vstart:1024],
                            pt[:, vstart:1024])
                    if k == 1 and pending is not None:
                        pending()
                        pending = None
                    if prev is not None:
                        emit_pv(*prev)
                    prev = (k, vstart, pt)
                    if g == QG - 1 and k == 8 * g + 4:
                        emit_half_epi(0)
                    drain(items, per_k)
                emit_pv(*prev)
                drain(items)
                if g == QG - 1:
                    emit_half_epi(1)
                    continue

                # epilogue part 1: denominator reduce + reciprocal (gates the
                # next group's scores via ps_s rotation)
                drt = ps_s.tile([P, 1024], f32, tag="s", name="drt")
                for hh in range(2):
                    nc.tensor.matmul(
                        drt[0:2, 512 * hh:512 * (hh + 1)], lhsT=ones_b,
                        rhs=acc_a[:, 512 * hh:512 * (hh + 1)],
                        start=True, stop=False)
                    nc.tensor.matmul(
                        drt[0:2, 512 * hh:512 * (hh + 1)], lhsT=ones_b,
                        rhs=acc_b[:, 512 * hh:512 * (hh + 1)],
                        start=False, stop=True)
                recipT = recipp.tile([1, 1024], f32, tag="recipT",
                                     name="recipT")
                nc.vector.reciprocal_approx_fast(recipT[:], drt[0:1, :])
                nc.gpsimd.dma_start(out=rscratch[g:g + 1, :], in_=recipT[:])
                rs = rscratch[g:g + 1, :]
                rs_b = bass.AP(tensor=rs.tensor, offset=rs.offset,
                               ap=[[0, P], rs.ap[-1]])
                recipB = recipp.tile([P, 1024], f32, tag="recipB",
                                     name="recipB")
                nc.gpsimd.dma_start(out=recipB[:], in_=rs_b)

                # epilogue part 2, deferred into the next group; the last
                # group runs it inline, split into halves so the output DMA
                # of half 0 overlaps the normalize of half 1
                def make_tail(g, o_ps, recipB):
                    def tail():
                        last = g == QG - 1
                        ocu = ocnp.tile([P, 1024], f32, tag="ocu", name="ocu")
                        ocn = ocnp.tile([P, 1024], f32, tag="ocn", name="ocn")
                        halves = (slice(0, 512), slice(512, 1024)) if last \
                            else (slice(0, 1024),)
                        eng = nc.vector if last else nc.gpsimd
                        for hs in halves:
                            nc.vector.tensor_copy(ocu[:, hs], o_ps[:, hs])
                            eng.tensor_mul(ocn[:, hs], ocu[:, hs],
                                           recipB[:, hs])
                            nc.sync.dma_start(
                                out=outT[:, 1024 * g + hs.start:
                                         1024 * g + hs.stop],
                                in_=ocn[:, hs])
                    return tail
                pending = make_tail(g, o_ps, recipB)
            if pending is not None:
                pending()

    nc.finalize()
    return nc


def _get_nc():
    global _NC
    if _NC is None:
        _NC = build_program()
    return _NC


def _prepare_in_maps(x, Wq, Wk, Wv):
    x = np.asarray(x, dtype=np.float32)
    xb = x.astype(ml_dtypes.bfloat16)
    aux = np.zeros((P, 258), dtype=np.float32)
    aux[:, 0:2] = 1.0
    aux[:, 2:130] = np.eye(P, dtype=np.float32)
    aux[:, 130:258] = np.triu(np.ones((P, P), dtype=np.float32))  # kv <= q
    return [
        {
            "xT": np.ascontiguousarray(xb[b].T),
            "Wq": np.asarray(Wq, dtype=np.float32).astype(ml_dtypes.bfloat16),
            "Wk": np.asarray(Wk, dtype=np.float32).astype(ml_dtypes.bfloat16),
            "Wv": np.asarray(Wv, dtype=np.float32).astype(ml_dtypes.bfloat16),
            "aux": aux,
        }
        for b in range(NCORES)
    ]


def kernel(x, Wq, Wk, Wv):
    assert x.shape == (B, T, C) and Wq.shape == (C, HS)
    nc = _get_nc()
    in_maps = _prepare_in_maps(x, Wq, Wk, Wv)
    res = run_bass_kernel_spmd(nc, in_maps, list(range(NCORES)))
    return np.stack([np.ascontiguousarray(res.results[b]["outT"].T)
                     for b in range(NCORES)])
